# revision 3
# baseline (speedup 1.0000x reference)
"""Trainium2 Bass kernel for nn_BertClassifier_77309411685.

Data-parallel over 8 NeuronCores: each core handles 256 samples of the
2048-sample batch; the small base linear and 12 expert heads are replicated.

Per-core device algorithm (B_c=256 samples, processed as 2 halves of 128):
  1. indirect-DMA gather: for each sample, the 8 consecutive embedding rows
     starting at `start` (spans are 1..8 long and start <= S-9, so 8 rows are
     always in bounds) -> [128, 8*768] tile, one sample per partition.
  2. masked mean over the span via fused DVE multiply-accumulate with
     per-partition weights (i < len) / len.
  3. two static context rows loaded with strided DMA.
  4. PE transposes build featT [3H=2304, 256] (feature-major) from the
     sample-major center/context tiles.
  5. base linear: hiddenT[inner, b] = relu(W_base @ feat + b_base) via 18
     K-chunk matmuls per 128-wide inner tile, bias+relu fused in the
     PSUM->SBUF activation.
  6. expert heads: compute all 12 experts at once, out36[b, e*3+n], with the
     bias folded in as an extra K=1 matmul against a ones row; then select
     the right expert per sample with an is_equal mask and a strided reduce.
"""

import numpy as np
from contextlib import ExitStack

import concourse.bass as bass
import concourse.tile as tile
from concourse import bacc, mybir
from concourse.bass import IndirectOffsetOnAxis
from concourse.bass_utils import run_bass_kernel_spmd
from concourse.masks import make_identity

F32 = mybir.dt.float32
I32 = mybir.dt.int32

B, S, H = 2048, 256, 768
INNER, NB_CTX, NB_EXPERTS, NB_LABELS = 256, 2, 12, 3
NCORES = 8
BC = B // NCORES           # 256 samples per core
F3H = (NB_CTX + 1) * H     # 2304
KC = F3H // 128            # 18 contraction chunks
NE = NB_EXPERTS * NB_LABELS  # 36
SPAN = 8                   # max span length; always safe to gather 8 rows

# The reference picks 2 static context positions host-side with this exact rng.
CTX_IDX = [int(v) for v in np.random.default_rng(seed=0).choice(np.arange(S), size=NB_CTX)]

# float32r runs the PE at full rate (vs 4x slower for plain fp32) but requires
# fp32r-rounded producers; plain fp32 is the correctness baseline.
MM_DT = mybir.dt.float32


def _build():
    nc = bacc.Bacc(
        "TRN2",
        target_bir_lowering=False,
        debug=False,
        enable_asserts=False,
        num_devices=NCORES,
    )
    emb = nc.dram_tensor("emb", [BC * S, H], F32, kind="ExternalInput").ap()
    pos = nc.dram_tensor("pos", [BC, 2], I32, kind="ExternalInput").ap()
    cat = nc.dram_tensor("cat", [BC, 1], I32, kind="ExternalInput").ap()
    wbT = nc.dram_tensor("wbT", [F3H, INNER], F32, kind="ExternalInput").ap()
    bb = nc.dram_tensor("bb", [INNER], F32, kind="ExternalInput").ap()
    wexpT = nc.dram_tensor("wexpT", [INNER + 1, NE], F32, kind="ExternalInput").ap()
    out = nc.dram_tensor("out", [BC, NB_LABELS], F32, kind="ExternalOutput").ap()

    emb3d = emb.rearrange("(b s) h -> b s h", s=S)

    def mm(x):
        return x.bitcast(MM_DT) if MM_DT != F32 else x

    with tile.TileContext(nc) as tc, ExitStack() as ctx:
        pool = ctx.enter_context(tc.tile_pool(name="main", bufs=1))
        gpool = ctx.enter_context(tc.tile_pool(name="gp", bufs=2))
        spool = ctx.enter_context(tc.tile_pool(name="small", bufs=2))
        pst = ctx.enter_context(tc.tile_pool(name="pst", bufs=4, space="PSUM"))
        psh = ctx.enter_context(tc.tile_pool(name="psh", bufs=2, space="PSUM"))
        ps36p = ctx.enter_context(tc.tile_pool(name="ps36p", bufs=2, space="PSUM"))

        identity = pool.tile([128, 128], F32)
        make_identity(nc, identity[:])

        # Replicated weights.
        # wbT_t[p, c*INNER + m] = W_base.T[c*128 + p, m]
        wbT_t = pool.tile([128, KC * INNER], F32)
        nc.sync.dma_start(
            wbT_t[:].rearrange("p (c m) -> p c m", m=INNER),
            wbT.rearrange("(c p) m -> p c m", p=128),
        )
        bb_t = pool.tile([128, 2], F32)  # bb_t[p, t] = b_base[t*128 + p]
        nc.sync.dma_start(bb_t[:], bb.rearrange("(t p) -> p t", p=128))
        wexpA = pool.tile([128, NE], F32)
        nc.sync.dma_start(wexpA[:], wexpT[0:128, :])
        wexpB = pool.tile([128, NE], F32)
        nc.sync.dma_start(wexpB[:], wexpT[128:256, :])
        wexpC = pool.tile([1, NE], F32)
        nc.sync.dma_start(wexpC[:], wexpT[256:257, :])
        ones1 = pool.tile([1, 256], F32)
        nc.vector.memset(ones1[:], 1.0)

        # Static per-partition iotas.
        io8 = pool.tile([128, SPAN], I32)
        nc.gpsimd.iota(io8[:], pattern=[[1, SPAN]], base=0, channel_multiplier=0)
        io8f = pool.tile([128, SPAN], F32)
        nc.vector.tensor_copy(io8f[:], io8[:])
        io36 = pool.tile([128, NE], I32)  # value = expert id e at column e*3+n
        nc.gpsimd.iota(io36[:], pattern=[[1, NB_EXPERTS], [0, NB_LABELS]], base=0,
                       channel_multiplier=0)
        io36f = pool.tile([128, NE], F32)
        nc.vector.tensor_copy(io36f[:], io36[:])

        featT = pool.tile([128, KC * 256], F32)
        catf_h = []

        for h in range(2):
            b0 = h * 128
            pos_t = spool.tile([128, 2], I32, tag="pos")
            nc.sync.dma_start(pos_t[:], pos[b0:b0 + 128, :])
            cat_t = spool.tile([128, 1], I32, tag="cat")
            nc.sync.dma_start(cat_t[:], cat[b0:b0 + 128, :])

            # Gather row index per sample: (b0 + p)*S + start_p.
            base_io = spool.tile([128, 1], I32, tag="baseio")
            nc.gpsimd.iota(base_io[:], pattern=[[1, 1]], base=b0 * S,
                           channel_multiplier=S)
            idx_t = spool.tile([128, 1], I32, tag="idx")
            nc.vector.tensor_tensor(out=idx_t[:], in0=base_io[:], in1=pos_t[:, 0:1],
                                    op=mybir.AluOpType.add)

            g = gpool.tile([128, SPAN * H], F32, tag="g")
            nc.gpsimd.indirect_dma_start(
                out=g[:], out_offset=None, in_=emb,
                in_offset=IndirectOffsetOnAxis(ap=idx_t[:, :1], axis=0),
            )

            ctx0 = gpool.tile([128, H], F32, tag="ctx0")
            nc.sync.dma_start(ctx0[:], emb3d[b0:b0 + 128, CTX_IDX[0], :])
            ctx1 = gpool.tile([128, H], F32, tag="ctx1")
            nc.sync.dma_start(ctx1[:], emb3d[b0:b0 + 128, CTX_IDX[1], :])

            # Span weights: w8[p, i] = (i < len_p) / len_p.
            len_i = spool.tile([128, 1], I32, tag="leni")
            nc.vector.tensor_tensor(out=len_i[:], in0=pos_t[:, 1:2], in1=pos_t[:, 0:1],
                                    op=mybir.AluOpType.subtract)
            len_f = spool.tile([128, 1], F32, tag="lenf")
            nc.vector.tensor_copy(len_f[:], len_i[:])
            rcp = spool.tile([128, 1], F32, tag="rcp")
            nc.vector.reciprocal(rcp[:], len_f[:])
            w8 = spool.tile([128, SPAN], F32, tag="w8")
            nc.vector.tensor_scalar(w8[:], io8f[:], len_f[:, :1], rcp[:, :1],
                                    op0=mybir.AluOpType.is_lt,
                                    op1=mybir.AluOpType.mult)

            # center[p, :] = sum_i w8[p, i] * g[p, i*H:(i+1)*H]
            center = gpool.tile([128, H], F32, tag="center")
            nc.vector.tensor_scalar(center[:], g[:, 0:H], w8[:, 0:1], None,
                                    op0=mybir.AluOpType.mult)
            for i in range(1, SPAN):
                nc.vector.scalar_tensor_tensor(
                    out=center[:], in0=g[:, i * H:(i + 1) * H], scalar=w8[:, i:i + 1],
                    in1=center[:], op0=mybir.AluOpType.mult, op1=mybir.AluOpType.add)

            # featT[c*256 + h*128 + b] columns via PE transposes of the three
            # sample-major sources (feat = [center | ctx0 | ctx1]).
            for si, src in enumerate((center, ctx0, ctx1)):
                for c in range(H // 128):
                    tp = pst.tile([128, 128], F32, tag="tp")
                    nc.tensor.transpose(tp[:], src[:, c * 128:(c + 1) * 128],
                                        identity[:])
                    col = (si * (H // 128) + c) * 256 + h * 128
                    nc.vector.tensor_copy(featT[:, col:col + 128], tp[:])

            catf = spool.tile([128, 1], F32, tag=f"catf{h}", bufs=1)
            nc.vector.tensor_copy(catf[:], cat_t[:])
            catf_h.append(catf)

        # hiddenT[p, mt*256 + (h*128 + b)] = relu(W_base @ feat + b_base)
        hiddenT = pool.tile([128, 2 * 256], F32)
        for mt in range(2):
            acc = psh.tile([128, 256], F32, tag="acc")
            for c in range(KC):
                nc.tensor.matmul(
                    acc[:],
                    lhsT=mm(wbT_t[:, c * INNER + mt * 128: c * INNER + (mt + 1) * 128]),
                    rhs=mm(featT[:, c * 256:(c + 1) * 256]),
                    start=(c == 0), stop=(c == KC - 1),
                )
            nc.scalar.activation(hiddenT[:, mt * 256:(mt + 1) * 256], acc[:],
                                 mybir.ActivationFunctionType.Relu,
                                 bias=bb_t[:, mt:mt + 1], scale=1.0)

        for h in range(2):
            b0 = h * 128
            ps36 = ps36p.tile([128, NE], F32, tag="ps36")
            nc.tensor.matmul(ps36[:], lhsT=mm(hiddenT[:, b0:b0 + 128]),
                             rhs=mm(wexpA[:]), start=True, stop=False)
            nc.tensor.matmul(ps36[:], lhsT=mm(hiddenT[:, 256 + b0:256 + b0 + 128]),
                             rhs=mm(wexpB[:]), start=False, stop=False)
            nc.tensor.matmul(ps36[:], lhsT=mm(ones1[:, b0:b0 + 128]),
                             rhs=mm(wexpC[:]), start=False, stop=True)

            mask36 = spool.tile([128, NE], F32, tag="mask36")
            nc.vector.tensor_scalar(mask36[:], io36f[:], catf_h[h][:, :1], None,
                                    op0=mybir.AluOpType.is_equal)
            prod = spool.tile([128, NE], F32, tag="prod")
            nc.vector.tensor_tensor(out=prod[:], in0=ps36[:], in1=mask36[:],
                                    op=mybir.AluOpType.mult)
            out3 = spool.tile([128, NB_LABELS], F32, tag="out3")
            nc.vector.tensor_reduce(
                out=out3[:],
                in_=prod[:].rearrange("p (e n) -> p n e", n=NB_LABELS),
                axis=mybir.AxisListType.X, op=mybir.AluOpType.add)
            nc.sync.dma_start(out[b0:b0 + 128, :], out3[:])

    nc.compile()
    return nc


_NC = None


def _get_nc():
    global _NC
    if _NC is None:
        _NC = _build()
    return _NC


def _prep_inputs(embeddings, position_indexes, categories, W_base, b_base,
                 W_experts, b_experts):
    emb = np.ascontiguousarray(np.asarray(embeddings, dtype=np.float32)).reshape(
        NCORES, BC * S, H)
    pos = np.ascontiguousarray(np.asarray(position_indexes).astype(np.int32)).reshape(
        NCORES, BC, 2)
    cat = np.ascontiguousarray(np.asarray(categories).astype(np.int32)).reshape(
        NCORES, BC, 1)
    wbT = np.ascontiguousarray(np.asarray(W_base, dtype=np.float32).T)  # [3H, INNER]
    bb = np.ascontiguousarray(np.asarray(b_base, dtype=np.float32))
    we = np.asarray(W_experts, dtype=np.float32)  # [12, 3, INNER]
    be = np.asarray(b_experts, dtype=np.float32)  # [12, 3]
    wexpT = np.concatenate(
        [we.transpose(2, 0, 1).reshape(INNER, NE), be.reshape(1, NE)], axis=0)
    wexpT = np.ascontiguousarray(wexpT)  # [INNER+1, 36]
    return [
        {"emb": emb[i], "pos": pos[i], "cat": cat[i], "wbT": wbT, "bb": bb,
         "wexpT": wexpT}
        for i in range(NCORES)
    ]


def _run(in_maps, **kw):
    nc = _get_nc()
    return run_bass_kernel_spmd(nc, in_maps, core_ids=list(range(NCORES)), **kw)


def kernel(embeddings, position_indexes, categories, W_base, b_base, W_experts,
           b_experts):
    in_maps = _prep_inputs(embeddings, position_indexes, categories, W_base,
                           b_base, W_experts, b_experts)
    res = _run(in_maps)
    return np.concatenate([r["out"] for r in res.results], axis=0)


# revision 8
# speedup vs baseline: 1.2234x; 1.2234x over previous
"""Trainium2 Bass kernel for nn_BertClassifier_77309411685.

Data-parallel over 8 NeuronCores: each core handles 256 samples of the
2048-sample batch; the small base linear and 12 expert heads are replicated.

Per-core device algorithm (B_c=256 samples, processed as 2 halves of 128):
  1. indirect-DMA gather: for each sample, the 8 consecutive embedding rows
     starting at `start` (spans are 1..8 long and start <= S-9, so 8 rows are
     always in bounds), split into two 4-row chunks so the masked-mean can
     start while the second chunk is still in flight. One sample per
     partition.
  2. masked mean over the span via fused DVE multiply-accumulate with
     per-partition weights (i < len) / len.
  3. two static context rows loaded with strided DMA.
  4. PE transposes build featT [3H=2304, 256] (feature-major) from the
     sample-major center/context tiles; 3 transposes share one PSUM bank and
     drain with a single strided copy.
  5. base linear: hiddenT[inner, b] = relu(W_base @ feat + b_base) via 18
     K-chunk matmuls per 128-wide inner tile, bias+relu fused in the
     PSUM->SBUF activation.
  6. expert heads: compute all 12 experts at once, out36[b, e*3+n], with the
     bias folded in as an extra K=1 matmul against a ones row; then select
     the right expert per sample with an is_equal mask and a strided reduce.

Constants (identity matrix, iota ramps, per-partition row bases) are shipped
as one small DRAM input instead of being built with gpsimd ops on device.
"""

import numpy as np
from contextlib import ExitStack

import concourse.bass as bass
import concourse.tile as tile
from concourse import bacc, mybir
from concourse.bass import IndirectOffsetOnAxis
from concourse.bass_utils import run_bass_kernel_spmd

F32 = mybir.dt.float32
I32 = mybir.dt.int32

B, S, H = 2048, 256, 768
INNER, NB_CTX, NB_EXPERTS, NB_LABELS = 256, 2, 12, 3
NCORES = 8
BC = B // NCORES           # 256 samples per core
F3H = (NB_CTX + 1) * H     # 2304
KC = F3H // 128            # 18 contraction chunks
NE = NB_EXPERTS * NB_LABELS  # 36
SPAN = 8                   # max span length; always safe to gather 8 rows
HC = H // 128              # 6 h-chunks per feature block

# The reference picks 2 static context positions host-side with this exact rng.
CTX_IDX = [int(v) for v in np.random.default_rng(seed=0).choice(np.arange(S), size=NB_CTX)]

# float32r runs the PE at full rate (vs 4x slower for plain fp32) but requires
# fp32r-rounded producers; plain fp32 is the correctness baseline.
MM_DT = mybir.dt.float32

# Const blob layout (f32 columns): identity [0:128), io8f [128:136),
# io36f [136:172).  Separate int32 blob: rowbase [p, 0] = p*S.
C_ID, C_IO8, C_IO36, C_NF = 0, 128, 136, 172


def _build():
    nc = bacc.Bacc(
        "TRN2",
        target_bir_lowering=False,
        debug=False,
        enable_asserts=False,
        num_devices=NCORES,
    )
    emb = nc.dram_tensor("emb", [BC * S, H], F32, kind="ExternalInput").ap()
    pos = nc.dram_tensor("pos", [BC, 2], I32, kind="ExternalInput").ap()
    cat = nc.dram_tensor("cat", [BC, 1], I32, kind="ExternalInput").ap()
    wbT = nc.dram_tensor("wbT", [F3H, INNER], F32, kind="ExternalInput").ap()
    bb = nc.dram_tensor("bb", [INNER], F32, kind="ExternalInput").ap()
    wexpT = nc.dram_tensor("wexpT", [INNER + 1, NE], F32, kind="ExternalInput").ap()
    cstf = nc.dram_tensor("cstf", [128, C_NF], F32, kind="ExternalInput").ap()
    # csti cols: 0,1 = (h*128+p)*S row base per half; 2 = constant 4
    csti = nc.dram_tensor("csti", [128, 3], I32, kind="ExternalInput").ap()
    out = nc.dram_tensor("out", [BC, NB_LABELS], F32, kind="ExternalOutput").ap()

    emb3d = emb.rearrange("(b s) h -> b s h", s=S)

    def mm(x):
        return x.bitcast(MM_DT) if MM_DT != F32 else x

    with tile.TileContext(nc) as tc, ExitStack() as ctx:
        pool = ctx.enter_context(tc.tile_pool(name="main", bufs=1))
        gpool = ctx.enter_context(tc.tile_pool(name="gp", bufs=2))
        spool = ctx.enter_context(tc.tile_pool(name="small", bufs=2))
        pst = ctx.enter_context(tc.tile_pool(name="pst", bufs=4, space="PSUM"))
        psh = ctx.enter_context(tc.tile_pool(name="psh", bufs=2, space="PSUM"))
        ps36p = ctx.enter_context(tc.tile_pool(name="ps36p", bufs=2, space="PSUM"))

        # --- phase 0: tiny front-of-queue loads the gather depends on ---
        cstf_t = pool.tile([128, C_NF], F32)
        nc.sync.dma_start(cstf_t[:], cstf[:, :])
        rowb = pool.tile([128, 3], I32)
        nc.sync.dma_start(rowb[:], csti[:, :])
        pos_t = pool.tile([128, 4], I32)  # [p, h*2 + j] = pos[h*128+p, j]
        nc.sync.dma_start(pos_t[:].rearrange("p (h j) -> p h j", j=2),
                          pos.rearrange("(h p) j -> p h j", p=128))
        cat_t = pool.tile([128, 2], I32)  # [p, h] = cat[h*128+p]
        nc.sync.dma_start(cat_t[:].rearrange("p (h j) -> p h j", j=1),
                          cat.rearrange("(h p) j -> p h j", p=128))

        identity = cstf_t[:, C_ID:C_ID + 128]
        io8f = cstf_t[:, C_IO8:C_IO8 + SPAN]
        io36f = cstf_t[:, C_IO36:C_IO36 + NE]

        # --- phase 1: per-half index chains + gathers, earliest possible ---
        g_chunks = []   # [h][chunk] -> tile [128, 4*H]
        w8_h = []
        for h in range(2):
            b0 = h * 128
            idx_t = spool.tile([128, 1], I32, tag=f"idx{h}", bufs=1)
            # idx = (h*128 + p)*S + start
            nc.vector.tensor_tensor(out=idx_t[:], in0=rowb[:, h:h + 1],
                                    in1=pos_t[:, 2 * h:2 * h + 1],
                                    op=mybir.AluOpType.add)
            idx2_t = spool.tile([128, 1], I32, tag=f"idx2{h}", bufs=1)
            nc.vector.tensor_tensor(out=idx2_t[:], in0=idx_t[:], in1=rowb[:, 2:3],
                                    op=mybir.AluOpType.add)
            ch = []
            for ci, it in enumerate((idx_t, idx2_t)):
                g = gpool.tile([128, 4 * H], F32, tag=f"g{h}{ci}", bufs=1)
                nc.gpsimd.indirect_dma_start(
                    out=g[:], out_offset=None, in_=emb,
                    in_offset=IndirectOffsetOnAxis(ap=it[:, :1], axis=0),
                )
                ch.append(g)
            g_chunks.append(ch)

            # span weights w8[p, i] = (i < len) / len
            len_i = spool.tile([128, 1], I32, tag=f"leni{h}", bufs=1)
            nc.vector.tensor_tensor(out=len_i[:], in0=pos_t[:, 2 * h + 1:2 * h + 2],
                                    in1=pos_t[:, 2 * h:2 * h + 1],
                                    op=mybir.AluOpType.subtract)
            len_f = spool.tile([128, 1], F32, tag=f"lenf{h}", bufs=1)
            nc.vector.tensor_copy(len_f[:], len_i[:])
            rcp = spool.tile([128, 1], F32, tag=f"rcp{h}", bufs=1)
            nc.vector.reciprocal(rcp[:], len_f[:])
            w8 = spool.tile([128, SPAN], F32, tag=f"w8{h}", bufs=1)
            nc.vector.tensor_scalar(w8[:], io8f, len_f[:, :1], rcp[:, :1],
                                    op0=mybir.AluOpType.is_lt,
                                    op1=mybir.AluOpType.mult)
            w8_h.append(w8)

        # --- phase 2: context rows + replicated weights (overlap gathers) ---
        ctxs = []
        for h in range(2):
            b0 = h * 128
            ctx0 = gpool.tile([128, H], F32, tag=f"ctx0{h}", bufs=1)
            nc.sync.dma_start(ctx0[:], emb3d[b0:b0 + 128, CTX_IDX[0], :])
            ctx1 = gpool.tile([128, H], F32, tag=f"ctx1{h}", bufs=1)
            nc.sync.dma_start(ctx1[:], emb3d[b0:b0 + 128, CTX_IDX[1], :])
            ctxs.append((ctx0, ctx1))

        wbT_t = pool.tile([128, KC * INNER], F32)
        nc.sync.dma_start(
            wbT_t[:].rearrange("p (c m) -> p c m", m=INNER),
            wbT.rearrange("(c p) m -> p c m", p=128),
        )
        bb_t = pool.tile([128, 2], F32)  # bb_t[p, t] = b_base[t*128 + p]
        nc.sync.dma_start(bb_t[:], bb.rearrange("(t p) -> p t", p=128))
        wexpA = pool.tile([128, NE], F32)
        nc.sync.dma_start(wexpA[:], wexpT[0:128, :])
        wexpB = pool.tile([128, NE], F32)
        nc.sync.dma_start(wexpB[:], wexpT[128:256, :])
        wexpC = pool.tile([1, NE], F32)
        nc.sync.dma_start(wexpC[:], wexpT[256:257, :])
        ones1 = pool.tile([1, 256], F32)
        nc.vector.memset(ones1[:], 1.0)

        # --- phase 3: masked mean + transposes into featT ---
        featT = pool.tile([128, KC * 256], F32)
        featT3 = featT[:].rearrange("p (si rest) -> p si rest", si=3)
        catf_h = []
        for h in range(2):
            center = gpool.tile([128, H], F32, tag=f"center{h}", bufs=1)
            w8 = w8_h[h]
            nc.vector.tensor_scalar(center[:], g_chunks[h][0][:, 0:H], w8[:, 0:1],
                                    None, op0=mybir.AluOpType.mult)
            for i in range(1, SPAN):
                g = g_chunks[h][i // 4]
                off = (i % 4) * H
                nc.vector.scalar_tensor_tensor(
                    out=center[:], in0=g[:, off:off + H], scalar=w8[:, i:i + 1],
                    in1=center[:], op0=mybir.AluOpType.mult, op1=mybir.AluOpType.add)

            # featT[:, si*1536 + c*256 + h*128 + b] via PE transposes; the three
            # sources share one PSUM tile per h-chunk, drained by one copy.
            ctx0, ctx1 = ctxs[h]
            for c in range(HC):
                tp = pst.tile([128, 3 * 128], F32, tag="tp")
                for si, src in enumerate((center, ctx0, ctx1)):
                    nc.tensor.transpose(tp[:, si * 128:(si + 1) * 128],
                                        src[:, c * 128:(c + 1) * 128], identity)
                col = c * 256 + h * 128
                eng = nc.vector if c % 2 == 0 else nc.scalar
                dst = featT3[:, :, col:col + 128]
                if eng is nc.scalar:
                    nc.scalar.copy(dst, tp[:].rearrange("p (si x) -> p si x", si=3))
                else:
                    nc.vector.tensor_copy(dst, tp[:].rearrange("p (si x) -> p si x", si=3))

            catf = spool.tile([128, 1], F32, tag=f"catf{h}", bufs=1)
            nc.vector.tensor_copy(catf[:], cat_t[:, h:h + 1])
            catf_h.append(catf)

        # --- phase 4: base linear -> hiddenT [inner, b], bias+relu fused ---
        hiddenT = pool.tile([128, 2 * 256], F32)
        for mt in range(2):
            acc = psh.tile([128, 256], F32, tag="acc")
            for c in range(KC):
                nc.tensor.matmul(
                    acc[:],
                    lhsT=mm(wbT_t[:, c * INNER + mt * 128: c * INNER + (mt + 1) * 128]),
                    rhs=mm(featT[:, c * 256:(c + 1) * 256]),
                    start=(c == 0), stop=(c == KC - 1),
                )
            nc.scalar.activation(hiddenT[:, mt * 256:(mt + 1) * 256], acc[:],
                                 mybir.ActivationFunctionType.Relu,
                                 bias=bb_t[:, mt:mt + 1], scale=1.0)

        # --- phase 5: expert heads + per-sample selection ---
        for h in range(2):
            b0 = h * 128
            ps36 = ps36p.tile([128, NE], F32, tag="ps36")
            nc.tensor.matmul(ps36[:], lhsT=mm(hiddenT[:, b0:b0 + 128]),
                             rhs=mm(wexpA[:]), start=True, stop=False)
            nc.tensor.matmul(ps36[:], lhsT=mm(hiddenT[:, 256 + b0:256 + b0 + 128]),
                             rhs=mm(wexpB[:]), start=False, stop=False)
            nc.tensor.matmul(ps36[:], lhsT=mm(ones1[:, b0:b0 + 128]),
                             rhs=mm(wexpC[:]), start=False, stop=True)

            mask36 = spool.tile([128, NE], F32, tag="mask36")
            nc.vector.tensor_scalar(mask36[:], io36f, catf_h[h][:, :1], None,
                                    op0=mybir.AluOpType.is_equal)
            prod = spool.tile([128, NE], F32, tag="prod")
            nc.vector.tensor_tensor(out=prod[:], in0=ps36[:], in1=mask36[:],
                                    op=mybir.AluOpType.mult)
            out3 = spool.tile([128, NB_LABELS], F32, tag="out3")
            nc.vector.tensor_reduce(
                out=out3[:],
                in_=prod[:].rearrange("p (e n) -> p n e", n=NB_LABELS),
                axis=mybir.AxisListType.X, op=mybir.AluOpType.add)
            nc.sync.dma_start(out[b0:b0 + 128, :], out3[:])

    nc.compile()
    return nc


_NC = None


def _get_nc():
    global _NC
    if _NC is None:
        _NC = _build()
    return _NC


def _const_blobs():
    cstf = np.zeros((128, C_NF), dtype=np.float32)
    cstf[:, C_ID:C_ID + 128] = np.eye(128, dtype=np.float32)
    cstf[:, C_IO8:C_IO8 + SPAN] = np.arange(SPAN, dtype=np.float32)[None, :]
    cstf[:, C_IO36:C_IO36 + NE] = np.repeat(
        np.arange(NB_EXPERTS, dtype=np.float32), NB_LABELS)[None, :]
    csti = np.zeros((128, 3), dtype=np.int32)
    csti[:, 0] = np.arange(128, dtype=np.int32) * S
    csti[:, 1] = (np.arange(128, dtype=np.int32) + 128) * S
    csti[:, 2] = 4
    return cstf, csti


def _prep_inputs(embeddings, position_indexes, categories, W_base, b_base,
                 W_experts, b_experts):
    emb = np.ascontiguousarray(np.asarray(embeddings, dtype=np.float32)).reshape(
        NCORES, BC * S, H)
    pos = np.ascontiguousarray(np.asarray(position_indexes).astype(np.int32)).reshape(
        NCORES, BC, 2)
    cat = np.ascontiguousarray(np.asarray(categories).astype(np.int32)).reshape(
        NCORES, BC, 1)
    wbT = np.ascontiguousarray(np.asarray(W_base, dtype=np.float32).T)  # [3H, INNER]
    bb = np.ascontiguousarray(np.asarray(b_base, dtype=np.float32))
    we = np.asarray(W_experts, dtype=np.float32)  # [12, 3, INNER]
    be = np.asarray(b_experts, dtype=np.float32)  # [12, 3]
    wexpT = np.concatenate(
        [we.transpose(2, 0, 1).reshape(INNER, NE), be.reshape(1, NE)], axis=0)
    wexpT = np.ascontiguousarray(wexpT)  # [INNER+1, 36]
    cstf, csti = _const_blobs()
    return [
        {"emb": emb[i], "pos": pos[i], "cat": cat[i], "wbT": wbT, "bb": bb,
         "wexpT": wexpT, "cstf": cstf, "csti": csti}
        for i in range(NCORES)
    ]


def _run(in_maps, **kw):
    nc = _get_nc()
    return run_bass_kernel_spmd(nc, in_maps, core_ids=list(range(NCORES)), **kw)


def kernel(embeddings, position_indexes, categories, W_base, b_base, W_experts,
           b_experts):
    in_maps = _prep_inputs(embeddings, position_indexes, categories, W_base,
                           b_base, W_experts, b_experts)
    res = _run(in_maps)
    return np.concatenate([r["out"] for r in res.results], axis=0)
